# revision 57
# baseline (speedup 1.0000x reference)
"""Trainium2 Bass kernel for PixelUnshuffle->MHA->PixelShuffle (nn_Attention).

Reference computation (per batch element, 8 batch elements data-parallel
across 8 NeuronCores):
  x [64, 256, 256] --PixelUnshuffle(8)--> tokens [N=1024, C=4096]
  qkv = tokens @ W_qkv            [1024, 768]
  4-head attention (d=64), softmax over tokens
  y = attn_out @ W_out + b_out    [1024, 4096]
  --PixelShuffle(8)--> [64, 256, 256]

Layout strategy (v3): ALL data reshuffling happens on the host. x is
pre-packed (and pre-cast to bf16) into the exact [w, cg, p, r2, hh, ww]
tile layout the QKV matmul consumes, so the kernel issues just 8 fully
contiguous 1 MB input DMAs and zero de-stride copies. The output is
written as raw [nq, ct, p, hq, ww, r2] tiles (16 contiguous 512 KB DMAs)
and pixel-shuffled + upcast to f32 on the host. DMA-issue instructions
(~0.6us of engine time each) were the stage-3 bottleneck before this.

Token index   n = hh*32 + ww            (hh, ww in [0,32))
Channel index c = c0*64 + r1*8 + r2     (c0 in [0,64), r1, r2 in [0,8))
partition p = (c0 % 16)*8 + r1 within a cg/ct block of 16 c0's

Performance structure:
 - W_qkv arrives in 16 half-chunks ordered exactly as the matmul loop
   consumes them; W_out is deferred to the second token window. PE warmup
   matmuls hold the HAM clock-gate at 2.4 GHz until real work arrives
   (the PE drops to 1.2 GHz after any ~3.4us idle window).
 - Attention is computed transposed (dotsT[m, n], summed token m on
   partitions): dotsT = kT (lhsT) x qT -> exp -> av, with a ones column in
   v accumulating the softmax denominator Z for free (row 64 of oaug).
   The mc loop is software-pipelined with av(mc-1) issued after dots(mc)
   so the in-order PE queue never stalls behind the scalar-engine exp
   stream (exp is the stage-2 floor: (N+352)/1.2 ns, scalar is the only
   exp-capable engine). The exp table is preloaded at kernel start.
 - 1/Z per (n-half, head-pair): [1,1024]->[64,16] SBUF redistribute DMA,
   reciprocal, DRAM round trip for a 0-stride partition broadcast
   (partition_broadcast is broken for nonzero base partitions; 0-stride
   partition APs are DRAM-source only), overlapping the next block.
 - Output projection accumulates all 8 r2 blocks of a (nq, ct) tile in
   one 4-bank PSUM tile (dt tag allocated first so stage-3 PSUM reuses
   the dts banks, which free early). outT is split per n-half so stage-3
   nq 0/1 only waits on the first half's normalize. Evacuation is a
   single strided-read/contiguous-bf16-write copy, alternating
   vector/scalar.
"""

import sys

if "/opt/trn_rl_repo" not in sys.path:
    sys.path.insert(0, "/opt/trn_rl_repo")

import os

import ml_dtypes
import numpy as np

import concourse.bass as bass
from concourse import bacc, mybir, tile
from concourse.bass_utils import run_bass_kernel_spmd

F32 = mybir.dt.float32
BF16 = mybir.dt.bfloat16

SCALE = 0.125  # DIM_HEAD ** -0.5

_CACHE = {}


def _build(debug_outs=False, zero_bias=False):
    nc = bacc.Bacc("TRN2", target_bir_lowering=False, debug=False, num_devices=8)

    # x pre-packed on host: [w*4+cg, p, r2*2048 + hh*32 + ww]
    x_d = nc.dram_tensor("x", [8, 128, 4096], BF16, kind="ExternalInput").ap()
    # W_qkv split on host into qk columns and v columns, each pre-packed in
    # exact SBUF chunk layout so every load is one fully contiguous DMA
    # (column-sliced loads would produce 512B packets that crawl); the v
    # columns are only consumed by the deferred v pass, cutting the
    # startup-critical DMA bytes from 10.3 to 8.2 MB
    wq_d = nc.dram_tensor("W_qkv", [8, 2, 128, 1024], BF16, kind="ExternalInput").ap()
    wqv_d = nc.dram_tensor("W_qkv_v", [8, 2, 128, 512], BF16, kind="ExternalInput").ap()
    wo_d = nc.dram_tensor("W_out", [256, 4096], BF16, kind="ExternalInput").ap()
    b_d = nc.dram_tensor("b_out", [4096], F32, kind="ExternalInput").ap()
    # raw output tiles: [(nq*4+ct)*2+half, p, r2l*256 + hq*32 + ww] with
    # r2 = half*4 + r2l; host pixel-shuffles
    out_d = nc.dram_tensor("out", [32, 128, 1024], BF16, kind="ExternalOutput").ap()

    zrc_d = nc.dram_tensor("zr_scratch", [4, 1024], F32).ap()

    dbg = None
    if debug_outs:
        dbg = {
            "qkT": nc.dram_tensor(
                "dbg_qkT", [128, 4, 1024], F32, kind="ExternalOutput"
            ).ap(),
            "v_sb": nc.dram_tensor(
                "dbg_v", [128, 8, 4, 68], F32, kind="ExternalOutput"
            ).ap(),
            "outT": nc.dram_tensor(
                "dbg_outT", [128, 2, 2, 512], F32, kind="ExternalOutput"
            ).ap(),
        }

    def dram_ap(base, off, pattern):
        return bass.AP(tensor=base.tensor, offset=base.offset + off, ap=pattern)

    with tile.TileContext(nc) as tc:
        _build_tiled(
            nc, tc, x_d, wq_d, wqv_d, wo_d, b_d, out_d, zrc_d, dram_ap, dbg, zero_bias
        )
    nc.compile()
    return nc


def _build_tiled(nc, tc, x_d, wq_d, wqv_d, wo_d, b_d, out_d, zrc_d, dram_ap, dbg=None, zero_bias=False):
    from contextlib import ExitStack

    with ExitStack() as ctx:
        pers = ctx.enter_context(tc.tile_pool(name="pers", bufs=1))
        s23 = ctx.enter_context(tc.tile_pool(name="s23", bufs=1))

        # ---- persistent tiles ----
        # qkT[d-part, ot, n] : ot 0,1 = q dims 0..128,128..256; ot 2,3 = k
        qkT = pers.tile([128, 4, 1024], BF16)
        # v_aug[m-part, mc, h, 68] bf16, col 64 = ones (65-67 pad for align)
        v_sb = pers.tile([128, 8, 4, 68], BF16)
        # outT[i-part, ic, n-half] split per nh for fine-grained stage-3 deps
        outT = [pers.tile([128, 2, 512], BF16, name=f"outT{nh}") for nh in range(2)]
        # bias[c-part, r2, cg]
        bias_sb = pers.tile([128, 8, 4], F32)
        # W_out tile in outer pool; DMA issued during window-1 staging
        wo_sb = s23.tile([128, 2, 4096], BF16)  # [i-part, ic, c_perm]

        nc.vector.memset(v_sb[:, :, :, 64:68], 1.0)
        # preload the exp activation table off the critical path
        et_in = pers.tile([64, 16], F32)
        et_out = pers.tile([64, 16], F32)
        nc.vector.memset(et_in[:], 0.0)
        nc.scalar.activation(
            et_out[:], et_in[:], mybir.ActivationFunctionType.Exp, scale=SCALE
        )

        # =========================== stage 1 ===========================
        # QKV projection; 2 windows of 512 tokens (hh-halves). x arrives
        # host-packed: one contiguous 1 MB DMA per (w, cg) tile.
        with (
            tc.tile_pool(name="wq", bufs=1) as wqp,
            tc.tile_pool(name="xw", bufs=1) as xwp,
            tc.tile_pool(name="ps1", bufs=1, space="PSUM") as ps1,
        ):
            # each (r2, cg-pair) weight chunk is its OWN tile: dependency
            # tracking is tile-granular, so a single big weight tile makes
            # the first matmul wait for ALL chunk DMAs (~22us) instead of
            # just its own (~11us)
            wqk_t = {}
            wqv_t = {}

            # PE warmup: dummy matmuls so HAM is at 2.4 GHz when the first
            # real matmul arrives (ends ~when the first data lands)
            warm = wqp.tile([128, 512], BF16)
            nc.vector.memset(warm[:], 0.0)
            warm_ps = ps1.tile([128, 512], F32, tag="qk0", bufs=1)
            for i in range(24):
                nc.tensor.matmul(
                    warm_ps[:], warm[:, 0:128], warm[:], start=True, stop=True
                )

            def load_wq_qk(r2, i, eng):
                # cg-pair chunk, fully contiguous in host-packed layout
                t = wqp.tile([128, 2, 512], BF16, name=f"wqk_{r2}_{i}")
                eng.dma_start(
                    out=t[:],
                    in_=dram_ap(
                        wq_d, (r2 * 2 + i) * 131072, [[1024, 128], [1, 1024]]
                    ),
                )
                wqk_t[(r2, i)] = t

            def load_wq_v(r2, i, eng):
                t = wqp.tile([128, 2, 256], BF16, name=f"wqv_{r2}_{i}")
                eng.dma_start(
                    out=t[:],
                    in_=dram_ap(
                        wqv_d, (r2 * 2 + i) * 65536, [[512, 128], [1, 512]]
                    ),
                )
                wqv_t[(r2, i)] = t

            def load_x(w, cg, eng):
                # bufs=8: every tile gets a fresh buffer, so no DMA ever
                # waits on a buffer-release semaphore (a waiting DMA blocks
                # its whole queue in-order) — deferral is purely by
                # emission position
                xtb = xwp.tile(
                    [128, 8, 16, 32], BF16, tag="xtb", bufs=8, name=f"xtb_{w}_{cg}"
                )
                eng.dma_start(
                    out=xtb[:],
                    in_=dram_ap(x_d, (w * 4 + cg) * 128 * 4096, [[4096, 128], [1, 4096]]),
                )
                return xtb

            # Startup staging: first x tile split across two queues (each
            # queue sustains only ~110 GB/s early), wq chunks in
            # consumption order on scalar/gpsimd behind it
            xtbs = {}
            xtb00 = xwp.tile([128, 8, 16, 32], BF16, tag="xtb", bufs=8, name="xtb_0_0")
            nc.sync.dma_start(
                out=xtb00[:, 0:4, :, :],
                in_=dram_ap(x_d, 0, [[4096, 128], [1, 2048]]),
            )
            nc.gpsimd.dma_start(
                out=xtb00[:, 4:8, :, :],
                in_=dram_ap(x_d, 2048, [[4096, 128], [1, 2048]]),
            )
            xtbs[(0, 0)] = xtb00
            for r2 in range(8):  # cg pair 0, consumption-ordered
                load_wq_qk(r2, 0, (nc.scalar, nc.gpsimd)[r2 % 2])
            xtbs[(0, 1)] = load_x(0, 1, nc.sync)
            for r2 in range(8):  # cg pair 1
                load_wq_qk(r2, 1, (nc.scalar, nc.gpsimd)[r2 % 2])
            xtbs[(0, 2)] = load_x(0, 2, nc.sync)
            xtbs[(0, 3)] = load_x(0, 3, nc.scalar)
            # host pre-arranges b_out as [p, r2, cg] so this is a flat copy
            nc.gpsimd.dma_start(
                out=bias_sb[:],
                in_=dram_ap(b_d, 0, [[32, 128], [4, 8], [1, 4]]),
            )
            # v-column weights queue behind all startup-critical loads;
            # they're consumed only from the w0 v pass (~27us after the
            # stream starts)
            for i in range(2):
                for r2 in range(8):
                    load_wq_v(r2, i, (nc.scalar, nc.gpsimd, nc.sync)[r2 % 3])

            wo_loaded = [False]

            for w in range(2):
                # 8 accumulation groups (4 qk + 4 v) live in 8 PSUM banks;
                # the window runs as a qk pass then a v pass so the
                # v-column weights stay off the startup-critical window and
                # the qkT evacuations overlap the v pass
                qks = [
                    ps1.tile([128, 512], F32, tag=f"qk{ot}", bufs=1, name=f"qk_{w}_{ot}")
                    for ot in range(4)
                ]
                vps = [
                    ps1.tile([128, 256], F32, tag=f"v{s}", bufs=1, name=f"v_{w}_{s}")
                    for s in range(4)
                ]
                for cg in range(4):
                    xtb = xtbs[(w, cg)]
                    for r2 in range(8):
                        first = cg == 0 and r2 == 0
                        last = cg == 3 and r2 == 7
                        for ot in range(4):
                            nc.tensor.matmul(
                                qks[ot][:],
                                wqk_t[(r2, cg // 2)][
                                    :, cg % 2, ot * 128 : (ot + 1) * 128
                                ],
                                xtb[:, r2, :, :],
                                start=first,
                                stop=last,
                            )
                # k evacuations (ot 2,3) first: stage-2 dots for m-chunks
                # 4-7 need them soonest; q(w1) is needed later (nh=1)
                for ot in (2, 3, 0, 1):
                    dst = qkT[:, ot, w * 512 : (w + 1) * 512]
                    if ot % 2 == 0:
                        nc.scalar.copy(dst, qks[ot][:])
                    else:
                        nc.vector.tensor_copy(dst, qks[ot][:])
                for cg in range(4):
                    xtb = xtbs.pop((w, cg))
                    for r2 in range(8):
                        first = cg == 0 and r2 == 0
                        last = cg == 3 and r2 == 7
                        for s in range(4):
                            nc.tensor.matmul(
                                vps[s][:],
                                xtb[:, r2, 4 * s : 4 * s + 4, :],
                                wqv_t[(r2, cg // 2)][:, cg % 2, :],
                                start=first,
                                stop=last,
                            )
                    # next window's tiles: fresh buffers (bufs=8), so these
                    # DMAs never wait — deferral is purely queue position
                    if w == 0:
                        nxt = (nc.sync, nc.scalar, nc.sync, nc.gpsimd)[cg]
                        xtbs[(1, cg)] = load_x(1, cg, nxt)
                        if not wo_loaded[0] and cg == 2:
                            wo_loaded[0] = True
                            for ic in range(2):
                                (nc.gpsimd, nc.scalar)[ic].dma_start(
                                    out=wo_sb[:, ic, :],
                                    in_=dram_ap(
                                        wo_d, ic * 524288, [[4096, 128], [1, 4096]]
                                    ),
                                )
                for s in range(4):
                    nc.vector.tensor_copy(
                        v_sb[:, 4 * w + s, :, 0:64],
                        vps[s][:].rearrange("p (h d) -> p h d", h=4),
                    )
                if w == 1:
                    # keep the PE busy through the ps1 pool-close barrier
                    # so HAM stays at 2.4 GHz into stage 2; serializes only
                    # behind the qk0 evacuation
                    ka = ps1.tile([128, 512], F32, tag="qk0", bufs=1, name="ka")
                    for i in range(8):
                        nc.tensor.matmul(
                            ka[:], warm[:, 0:128], warm[:], start=True, stop=True
                        )

        if dbg is not None:
            nc.gpsimd.dma_start(out=dbg["qkT"][:], in_=qkT[:])
            nc.gpsimd.dma_start(out=dbg["v_sb"][:], in_=v_sb[:])

        # ======================= stage 2: attention =======================
        # Loops: n-half (nh) outer, head-pair (hp), summed-chunk (mc) inner.
        # PE issue order pipelines: av(mc-1) goes after dots(mc) so the PE
        # always has ready work while the scalar engine streams exps.
        with (
            tc.tile_pool(name="s2", bufs=1) as s2,
            tc.tile_pool(name="psA", bufs=1, space="PSUM") as psA,
        ):
            # allocate the dt tag FIRST so it lands in the low PSUM banks;
            # stage-3's first y tile then reuses dts banks (freed early)
            # rather than oaug banks (freed only after the last normalize)
            dts0 = psA.tile([128, 2, 512], F32, tag="dt", bufs=2, name="dt_first")
            for nh in range(2):
                for hp in range(2):
                    oaug = [
                        psA.tile(
                            [128, 512], F32, tag=f"oa{h2}", bufs=2,
                            name=f"oaug_{nh}_{hp}_{h2}",
                        )
                        for h2 in range(2)
                    ]
                    ed_q = []
                    for mc in range(9):
                        if mc < 8:
                            if dts0 is not None:
                                dts, dts0 = dts0, None
                            else:
                                dts = psA.tile(
                                    [128, 2, 512], F32, tag="dt", bufs=2,
                                    name=f"dt_{nh}_{hp}_{mc}",
                                )
                            for h2 in range(2):
                                b = h2 * 64
                                nc.tensor.matmul(
                                    dts[:, h2, :],
                                    qkT[b : b + 64, 2 + hp, mc * 128 : (mc + 1) * 128],
                                    qkT[b : b + 64, hp, nh * 512 : (nh + 1) * 512],
                                    start=True,
                                    stop=True,
                                )
                            ed = s2.tile(
                                [128, 2, 512], BF16, tag="ed", bufs=4,
                                name=f"ed_{nh}_{hp}_{mc}",
                            )
                            nc.scalar.activation(
                                ed[:].rearrange("p a b -> p (a b)"),
                                dts[:].rearrange("p a b -> p (a b)"),
                                mybir.ActivationFunctionType.Exp,
                                scale=SCALE,
                            )
                            ed_q.append(ed)
                        if mc >= 1:
                            edp = ed_q[mc - 1]
                            for h2 in range(2):
                                h = 2 * hp + h2
                                nc.tensor.matmul(
                                    oaug[h2][0:68, :],
                                    v_sb[:, mc - 1, h, :],
                                    edp[:, h2, :],
                                    start=(mc == 1),
                                    stop=(mc == 8),
                                )

                    # ---- normalize this (nh, hp) block: out *= 1/Z ----
                    # oaug is first evacuated to SBUF so the PSUM banks (and
                    # the psA pool at scope close) free right after the last
                    # av matmul rather than after the Z DMA round trip.
                    # Normalize tiles live in the outer s23 pool for the
                    # same reason. Z = row 64 of oev.
                    slot = nh * 2 + hp
                    oev = s23.tile([65, 2, 512], F32, tag="oev", bufs=2)
                    for h2 in range(2):
                        # the last block's chain gates the psA pool close;
                        # split it vector || scalar (scalar's exps are done)
                        if nh == 1 and hp == 1 and h2 == 1:
                            nc.scalar.copy(oev[:, h2, :], oaug[h2][0:65, :])
                        else:
                            nc.vector.tensor_copy(oev[:, h2, :], oaug[h2][0:65, :])
                    z64 = s23.tile([64, 16], F32, tag="z64", bufs=2)
                    nc.sync.dma_start(out=z64[:], in_=oev[64:65, :, :])
                    z64r = s23.tile([64, 16], F32, tag="z64r", bufs=2)
                    nc.vector.reciprocal(z64r[:], z64[:])
                    nc.sync.dma_start(
                        out=zrc_d[slot, :].rearrange("(a b) -> a b", a=64),
                        in_=z64r[:],
                    )
                    for h2 in range(2):
                        zbc = s23.tile([64, 512], F32, tag=f"zbc{h2}", bufs=2)
                        nc.sync.dma_start(
                            out=zbc[:],
                            in_=dram_ap(
                                zrc_d, slot * 1024 + h2 * 512, [[0, 64], [1, 512]]
                            ),
                        )
                        if h2 == 0:
                            nc.vector.tensor_mul(
                                outT[nh][0:64, hp, :],
                                oev[0:64, h2, :],
                                zbc[:],
                            )
                        else:
                            onrm = s23.tile([64, 512], BF16, tag="onrm", bufs=2)
                            nc.vector.tensor_mul(onrm[:], oev[0:64, h2, :], zbc[:])
                            nc.sync.dma_start(
                                out=outT[nh][64:128, hp, :],
                                in_=onrm[:],
                            )

        if dbg is not None:
            for nh in range(2):
                nc.gpsimd.dma_start(out=dbg["outT"][:, :, nh, :], in_=outT[nh][:])

        # ---------------- stage 3: output projection ----------------
        # Each (nq, ct) tile is TWO independent 2-bank PSUM tiles (r2 0-3 /
        # 4-7) so evacuation of the first half starts at the tile midpoint
        # instead of trailing all 16 matmuls (the y_ps rotation stalled the
        # PE otherwise). Evacuation is an identity contiguous f32->bf16
        # copy (host absorbs the layout), the output DMA a fully
        # contiguous 256 KB transfer per half.
        with (
            tc.tile_pool(name="s3b", bufs=1) as s3,
            tc.tile_pool(name="ps3", bufs=1, space="PSUM") as ps3,
        ):
            for nq in range(4):
                for ct in range(4):
                    for half in range(2):
                        y_ps = ps3.tile(
                            [128, 4, 256], F32, tag=f"yps{half}", bufs=2,
                            name=f"yps_{nq}_{ct}_{half}",
                        )
                        for r2l in range(4):
                            r2 = half * 4 + r2l
                            for ic in range(2):
                                nc.tensor.matmul(
                                    y_ps[:, r2l, :],
                                    wo_sb[
                                        :,
                                        ic,
                                        r2 * 512
                                        + ct * 128 : r2 * 512
                                        + (ct + 1) * 128,
                                    ],
                                    outT[nq // 2][
                                        :, ic, (nq % 2) * 256 : (nq % 2 + 1) * 256
                                    ],
                                    start=(r2l % 2 == 0 and ic == 0),
                                    stop=(r2l % 2 == 1 and ic == 1),
                                )
                        y_t = s3.tile(
                            [128, 4, 256], BF16, tag=f"yt{half}", bufs=6,
                            name=f"yt_{nq}_{ct}_{half}",
                        )
                        if zero_bias:
                            # gpsimd has no PSUM port: vector/scalar split
                            if half == 0:
                                nc.vector.tensor_copy(y_t[:], y_ps[:])
                            else:
                                nc.scalar.copy(y_t[:], y_ps[:])
                        else:
                            bias_bc = bias_sb[
                                :, half * 4 : (half + 1) * 4, ct
                            ][:, :, None].broadcast_to([128, 4, 256])
                            nc.vector.tensor_add(y_t[:], y_ps[:], bias_bc)
                        if nq == 3:
                            # final tiles: split DMAs across queues so the
                            # end-of-kernel drain isn't one 256KB transfer
                            for qq in range(2):
                                deng = (nc.sync, nc.gpsimd, nc.scalar)[
                                    (ct * 4 + half * 2 + qq) % 3
                                ]
                                deng.dma_start(
                                    out=dram_ap(
                                        out_d,
                                        ((nq * 4 + ct) * 2 + half) * 128 * 1024
                                        + qq * 512,
                                        [[1024, 128], [1, 512]],
                                    ),
                                    in_=y_t[:, 2 * qq : 2 * qq + 2, :],
                                )
                        else:
                            deng = (nc.sync, nc.gpsimd, nc.scalar)[
                                (nq * 8 + ct * 2 + half) % 3
                            ]
                            deng.dma_start(
                                out=dram_ap(
                                    out_d,
                                    ((nq * 4 + ct) * 2 + half) * 128 * 1024,
                                    [[1024, 128], [1, 1024]],
                                ),
                                in_=y_t[:],
                            )


def _get_nc(zero_bias=False):
    key = f"nc_zb{int(zero_bias)}"
    if key not in _CACHE:
        _CACHE[key] = _build(zero_bias=zero_bias)
    return _CACHE[key]


def _prep_weights(W_qkv, W_out, b_out):
    wq_perm = (
        W_qkv.reshape(64, 8, 8, 768).transpose(2, 0, 1, 3).reshape(4096, 768)
    )
    # split qk vs v columns and pack each in exact SBUF chunk layout
    # [r2, i(cg pair), p, (cgl, cols)] so every device load is one fully
    # contiguous DMA: rows within an (r2, i) chunk are (cgl*128 + p)
    def pack(cols):
        n = cols.shape[1]
        t = cols.reshape(8, 2, 2, 128, n)       # [r2, i, cgl, p, o]
        t = t.transpose(0, 1, 3, 2, 4)          # [r2, i, p, cgl, o]
        return np.ascontiguousarray(
            t.reshape(8, 2, 128, 2 * n)
        ).astype(ml_dtypes.bfloat16)

    wq_qk = pack(wq_perm[:, 0:512])
    wq_v = pack(wq_perm[:, 512:768])
    wo_perm = np.ascontiguousarray(
        W_out.reshape(256, 64, 8, 8).transpose(0, 3, 1, 2).reshape(256, 4096)
    ).astype(ml_dtypes.bfloat16)
    # b_perm[r2*512 + c0*8 + r1] = b_out[c0*64 + r1*8 + r2], then laid out
    # [p, r2, cg] where p = (c0 % 16)*8 + r1, cg = c0 // 16
    b_perm = b_out.reshape(64, 8, 8).transpose(2, 0, 1).reshape(4096)
    b_perm = np.ascontiguousarray(
        b_perm.reshape(8, 4, 128).transpose(2, 0, 1).reshape(4096)
    ).astype(np.float32)
    return wq_qk, wq_v, wo_perm, b_perm


def _pack_x(xb):
    # xb [64, 256, 256] f32 -> [w*4+cg, p=(c0%16)*8+r1, r2*2048+hh*32+ww] bf16
    # x[c0, (w*16+hh)*8 + r1, ww*8 + r2]
    t = xb.astype(ml_dtypes.bfloat16)
    t = t.reshape(4, 16, 2, 16, 8, 32, 8)  # [cg, c0l, w, hh, r1, ww, r2]
    t = t.transpose(2, 0, 1, 4, 6, 3, 5)   # [w, cg, c0l, r1, r2, hh, ww]
    return np.ascontiguousarray(t.reshape(8, 128, 4096))


def _unpack_out(raw):
    # raw [32, 128, 1024] = [(nq*4+ct)*2+half, (c0%16)*8+r1, r2l*256+hq*32+ww]
    # with r2 = half*4 + r2l -> y[c0, (nq*8+hq)*8 + r1, ww*8 + r2]
    t = np.asarray(raw).reshape(4, 4, 2, 16, 8, 4, 8, 32)
    # [nq, ct, half, c0l, r1, r2l, hq, ww]
    t = t.transpose(1, 3, 0, 6, 4, 7, 2, 5)  # [ct, c0l, nq, hq, r1, ww, half, r2l]
    return t.reshape(64, 256, 256)


def kernel(x, W_qkv, W_out, b_out):
    nc = _get_nc(zero_bias=not np.any(np.asarray(b_out)))
    wq_qk, wq_v, wo_perm, b_perm = _prep_weights(
        np.asarray(W_qkv, dtype=np.float32),
        np.asarray(W_out, dtype=np.float32),
        np.asarray(b_out, dtype=np.float32),
    )

    in_maps = [
        {
            "x": _pack_x(np.asarray(x[b], dtype=np.float32)),
            "W_qkv": wq_qk,
            "W_qkv_v": wq_v,
            "W_out": wo_perm,
            "b_out": b_perm,
        }
        for b in range(8)
    ]
    trace = bool(int(os.environ.get("BENCH_TRACE", "0")))
    if trace:
        try:  # tracing needs the NTFF hook shim (see test.py); degrade if absent
            from antenv.axon_hooks import get_axon_ntff_profile_hook  # noqa: F401
        except ImportError:
            trace = False
    res = run_bass_kernel_spmd(nc, in_maps, core_ids=list(range(8)), trace=trace)
    if trace:
        _CACHE["last_result"] = res
    return np.stack(
        [_unpack_out(res.results[b]["out"]) for b in range(8)]
    ).astype(np.float32)
